# revision 30
# baseline (speedup 1.0000x reference)
"""Trainium2 Bass kernel for nn_BinaryMLP (binary MLP with BatchNorm, 3 hidden
layers + linear head), distributed data-parallel over 8 NeuronCores.

Math per hidden layer (reference):
    h = sign(a_prev) @ sign(W).T          # [B, H], exact integers in fp32
    h = g * (h - mean) / sqrt(var + eps) + b   # batch stats over FULL batch
    a = sign(h)

Since sign() only cares about the side of a per-feature affine threshold,
BN+sign folds into  a = Sign(g * h + (b*sqrt(var+eps) - g*mean))  computed on
the ACT engine with per-partition (per-feature) scale/bias. All matmul
operands are in {-1, 0, +1}  ->  fp8e4 operands with fp32 PSUM accumulation
are EXACT. Batch mean = (exact integer sum) / 8192 is exact in fp32, so the
whole network is bit-exact vs the fp32 jax reference.

Sharding: batch 8192 -> 1024 rows per core, weights replicated. Activations
live on-chip transposed as [feature(part), batch(free)] so BN stats are
free-axis reductions and the per-feature threshold is a per-partition scalar.
Cross-core BN stats via a 32KB DRAM AllReduce per layer.
"""

import numpy as np
import ml_dtypes

P = 128
N_CORES = 8
BN_EPS = 1e-5

FP8 = ml_dtypes.float8_e4m3


def build_kernel(
    b_c: int,  # batch rows per core
    d: int,  # input features (= contraction dim of layer 1)
    h: int,  # hidden features
    c_pad: int,  # output features padded to a multiple of 128
    n_cores: int = N_CORES,
    iters: int = 1,
    n_free: int = 512,  # matmul moving free dim (PSUM bank)
    skip_collective: bool = False,  # timing experiments only (wrong results)
    chunks: tuple = (16, 28),  # feature-tile boundaries for stats/AR/sign
    psum_bufs: int = 8,
    w_bufs: int = 5,
):
    """Build + compile the SPMD Bass kernel. Returns the compiled Bacc."""
    import concourse.bass as bass
    import concourse.mybir as mybir
    import concourse.tile as tile
    from concourse import bacc
    from concourse.bass import ds, ts

    f32 = mybir.dt.float32
    fp16 = mybir.dt.float16
    fp8 = mybir.dt.float8e4
    Act = mybir.ActivationFunctionType
    Alu = mybir.AluOpType

    ko_n = d // P  # k-tiles layer 1
    kh_n = h // P  # k-tiles layers 2/3 and head
    f_n = h // P  # hidden feature tiles
    fo_n = c_pad // P  # head feature tiles
    nb_n = b_c // n_free  # batch blocks per core
    assert d % P == 0 and h % P == 0 and c_pad % P == 0 and b_c % n_free == 0
    inv_b = 1.0 / (b_c * n_cores)  # power of two -> exact fp32 scaling

    nc = bacc.Bacc(
        "TRN2", target_bir_lowering=False, debug=False, num_devices=n_cores
    )

    a0_d = nc.dram_tensor("a0", [P, ko_n, b_c], fp8, kind="ExternalInput")
    w_d = [
        nc.dram_tensor(f"w{l + 1}", [f_n, P, (ko_n if l == 0 else kh_n), P], fp8,
                       kind="ExternalInput")
        for l in range(3)
    ]
    wo_d = nc.dram_tensor("wo", [fo_n, P, kh_n, P], fp8, kind="ExternalInput")
    # (g1,b1,g2,b2,g3,b3) packed [6, P, f_n]
    gb_d = nc.dram_tensor("gb", [6, P, f_n], f32, kind="ExternalInput")
    out_d = nc.dram_tensor("outT", [P, fo_n, b_c], f32, kind="ExternalOutput")

    with tile.TileContext(nc) as tc:
        with (
            tc.tile_pool(name="acts", bufs=2) as acts,  # fp8 activations (ping/pong)
            tc.tile_pool(name="hbuf", bufs=1) as hbuf,  # fp16 pre-BN values
            tc.tile_pool(name="wpool", bufs=w_bufs) as wpool,
            tc.tile_pool(name="psum", bufs=psum_bufs, space="PSUM") as psum,
            tc.tile_pool(name="stats", bufs=4) as stats,
            tc.tile_pool(name="scratch", bufs=2) as scratch,
            tc.tile_pool(name="consts", bufs=1) as consts,
            tc.tile_pool(name="dram", bufs=4, space="DRAM") as dram,
        ):
            gb_sb = consts.tile([P, 6, f_n], f32)
            nc.sync.dma_start(gb_sb[:], gb_d.ap().rearrange("l p o -> p l o"))
            eps_t = consts.tile([P, 1], f32)
            nc.vector.memset(eps_t[:], BN_EPS)

            chunk_bounds = []
            f_start = 0
            for f_end in [c for c in chunks if 0 < c < f_n] + [f_n]:
                chunk_bounds.append((f_start, f_end))
                f_start = f_end

            def emit_pairs(ps, w_tile, segs, nb, seg_lo, seg_hi, first, last):
                """Emit DoubleRow matmul pairs for activation segments
                [seg_lo, seg_hi). segs: list of (tile, ko_off, ko_len)."""
                n_pairs = sum(kl for _, _, kl in segs[seg_lo:seg_hi]) // 2
                j = 0
                for a_seg, ko_off, ko_len in segs[seg_lo:seg_hi]:
                    for k2 in range(ko_len // 2):
                        nc.tensor.matmul(
                            ps,
                            lhsT=w_tile[:, ds(ko_off + 2 * k2, 2), :],
                            rhs=a_seg[:, ts(k2, 2), ds(nb * n_free, n_free)],
                            start=(first and j == 0),
                            stop=(last and j == n_pairs - 1),
                            perf_mode=mybir.MatmulPerfMode.DoubleRow,
                        )
                        j += 1

            def gen_matmuls(w_dram_t, f_list, segs, sink, mid=None,
                            wave=False, extra_dmas=None):
                """Emit matmul groups for feature tiles in f_list contracting
                over activation segments `segs`. With wave=True the first 8
                psum groups are emitted as a wave: all-but-last-segment
                partial products first, then `mid()` (the previous chunk's
                deferred AllReduce-readback + Sign work, which produces the
                last segment), then the last-segment products. Emission order
                tracks data-readiness order so Tile's static per-engine
                schedule never traps ready work behind blocked work."""
                wave_f = f_list[: 8 // nb_n] if (wave and len(segs) > 1) else []
                rest_f = [f for f in f_list if f not in wave_f]
                k_n = sum(kl for _, _, kl in segs)

                wtiles, pss = {}, {}
                for i, f in enumerate(wave_f):
                    w_tile = wpool.tile([P, k_n, P], fp8, tag="w")
                    nc.sync.dma_start(w_tile[:], w_dram_t[f])
                    if i == 0 and extra_dmas:
                        extra_dmas()
                        extra_dmas = None
                    wtiles[f] = w_tile
                    for nb in range(nb_n):
                        pss[(f, nb)] = psum.tile(
                            [P, n_free], f32, tag="ps", name=f"ps_w{f}_{nb}"
                        )
                for f in wave_f:
                    for nb in range(nb_n):
                        emit_pairs(pss[(f, nb)], wtiles[f], segs, nb,
                                   0, len(segs) - 1, first=True, last=False)
                if mid is not None:
                    mid()
                    mid = None
                for f in wave_f:
                    for nb in range(nb_n):
                        emit_pairs(pss[(f, nb)], wtiles[f], segs, nb,
                                   len(segs) - 1, len(segs), first=False,
                                   last=True)
                        sink(f, nb, pss[(f, nb)])
                for i, f in enumerate(rest_f):
                    w_tile = wpool.tile([P, k_n, P], fp8, tag="w")
                    nc.sync.dma_start(w_tile[:], w_dram_t[f])
                    if i == 0 and extra_dmas:
                        extra_dmas()
                        extra_dmas = None
                    if i == 0 and mid is not None:
                        mid()
                        mid = None
                    for nb in range(nb_n):
                        ps = psum.tile([P, n_free], f32, tag="ps")
                        emit_pairs(ps, w_tile, segs, nb, 0, len(segs),
                                   first=True, last=True)
                        sink(f, nb, ps)
                if mid is not None:
                    mid()

            def hidden_layer(l, segs, pending=None, extra_dmas=None):
                """Returns (out_segs, pending). Each chunk's epilogue is split
                into partA (stats reduce + AllReduce trigger, emitted right
                after the chunk's matmuls) and partB (readback + thresholds +
                Signs, emitted later — interleaved into subsequent matmul
                emission so every engine's static order matches readiness
                order). The final chunk's partB is returned as `pending` and
                is emitted inside the NEXT layer's first matmul wave."""
                h_sb = hbuf.tile([P, f_n, b_c], fp16, tag="h")
                g_ap = gb_sb[:, 2 * l, :]
                b_ap = gb_sb[:, 2 * l + 1, :]
                out_segs = []
                accs = {}

                def chunk_partA(ci, f0, f1):
                    csz = f1 - f0
                    sum_acc, sq_acc = accs[ci]
                    # local stats -> AllReduce (trigger only)
                    stat_sb = stats.tile([P, 2 * csz], f32, tag="stat_sb")
                    nc.vector.tensor_reduce(
                        stat_sb[:, 0:csz], sum_acc[:, f0:f1, :],
                        mybir.AxisListType.X, Alu.add,
                    )
                    nc.vector.tensor_reduce(
                        stat_sb[:, csz : 2 * csz], sq_acc[:, f0:f1, :],
                        mybir.AxisListType.X, Alu.add,
                    )
                    if skip_collective:
                        return stat_sb
                    cc_in = dram.tile([P, 2 * csz], f32, tag="cc_in")
                    cc_out = dram.tile([P, 2 * csz], f32, tag="cc_out")
                    nc.gpsimd.dma_start(cc_in[:], stat_sb[:])
                    nc.gpsimd.collective_compute(
                        "AllReduce",
                        Alu.add,
                        replica_groups=[list(range(n_cores))],
                        ins=[cc_in.opt()],
                        outs=[cc_out.opt()],
                    )
                    return cc_out

                def make_partB(ci, f0, f1, ar_out):
                    """Returns (head, [sign_fn...]): head does the AllReduce
                    readback + threshold math; each sign_fn emits one feature
                    tile's Sign. Emitted piecemeal between later feature
                    tiles so ACT never has a long blocked burst queued ahead
                    of PSUM-recycling copies."""
                    csz = f1 - f0
                    a_out = acts.tile([P, csz, b_c], fp8, tag=f"act{ci}")
                    out_segs.append((a_out, f0, csz))
                    cvec = stats.tile([P, csz], f32, tag=f"cvec{ci}",
                                      name=f"cvec_{l}_{ci}")

                    def head():
                        if skip_collective:
                            gstat = ar_out
                        else:
                            gstat = stats.tile([P, 2 * csz], f32, tag="gstat")
                            nc.sync.dma_start(gstat[:], ar_out[:])
                        # threshold: a = Sign(g*h + (b*std - g*mean))
                        mean_t = stats.tile([P, csz], f32, tag="mean_t")
                        var_t = stats.tile([P, csz], f32, tag="var_t")
                        std_t = stats.tile([P, csz], f32, tag="std_t")
                        tmp_t = stats.tile([P, csz], f32, tag="tmp_t")
                        nc.vector.tensor_scalar_mul(
                            mean_t[:], gstat[:, 0:csz], inv_b
                        )
                        nc.vector.tensor_scalar_mul(
                            tmp_t[:], gstat[:, csz : 2 * csz], inv_b
                        )
                        nc.vector.tensor_tensor(
                            var_t[:], mean_t[:], mean_t[:], Alu.mult
                        )
                        nc.vector.tensor_tensor(
                            var_t[:], tmp_t[:], var_t[:], Alu.subtract
                        )
                        nc.scalar.activation(
                            std_t[:], var_t[:], Act.Sqrt, bias=eps_t[:]
                        )
                        nc.vector.tensor_tensor(
                            tmp_t[:], b_ap[:, f0:f1], std_t[:], Alu.mult
                        )
                        nc.vector.tensor_tensor(
                            std_t[:], g_ap[:, f0:f1], mean_t[:], Alu.mult
                        )
                        nc.vector.tensor_tensor(
                            cvec[:], tmp_t[:], std_t[:], Alu.subtract
                        )

                    def sign_of(f):
                        def emit():
                            nc.scalar.activation(
                                a_out[:, f - f0, :], h_sb[:, f, :], Act.Sign,
                                bias=cvec[:, f - f0 : f - f0 + 1],
                                scale=g_ap[:, f : f + 1],
                            )
                        return emit

                    return head, [sign_of(f) for f in range(f0, f1)]

                def sink(f, nb, ps):
                    ci = next(i for i, (lo, hi) in enumerate(chunk_bounds)
                              if lo <= f < hi)
                    sum_acc, sq_acc = accs[ci]
                    # ACT: copy to fp16 h (exact) + per-feature batch sum
                    nc.scalar.activation(
                        h_sb[:, f, ts(nb, n_free)], ps, Act.Copy,
                        accum_out=sum_acc[:, f, nb : nb + 1],
                    )
                    # DVE: square from the fp16 copy, then sum
                    # (only one PSUM operand allowed per DVE op)
                    hh = h_sb[:, f, ts(nb, n_free)]
                    sq_scr = scratch.tile([P, n_free], f32, tag="sq_scr")
                    nc.vector.tensor_tensor(sq_scr[:], hh, hh, Alu.mult)
                    nc.vector.tensor_reduce(
                        sq_acc[:, f, nb : nb + 1], sq_scr[:],
                        mybir.AxisListType.X, Alu.add,
                    )

                for ci in range(len(chunk_bounds)):
                    accs[ci] = (
                        stats.tile([P, f_n, nb_n], f32, tag="sum_acc",
                                   name=f"sum_acc_{l}_{ci}"),
                        stats.tile([P, f_n, nb_n], f32, tag="sq_acc",
                                   name=f"sq_acc_{l}_{ci}"),
                    )

                w_ap = w_d[l].ap()

                def gen_f(f_lo, f_hi, **kw):
                    gen_matmuls(w_ap, list(range(f_lo, f_hi)), segs, sink,
                                **kw)

                def chunk_close(ci, f0, f1, cover_hi):
                    """partA for chunk ci, then its partB spread over feature
                    tiles [f1, cover_hi): 2 ftiles of matmul cover while the
                    AllReduce flies, then Signs trickled between the rest."""
                    ar = chunk_partA(ci, f0, f1)
                    gen_f(f1, min(f1 + 2, cover_hi))
                    head, sign_fns = make_partB(ci, f0, f1, ar)
                    head()
                    rem = list(range(min(f1 + 2, cover_hi), cover_hi))
                    per = -(-len(sign_fns) // max(1, len(rem)))
                    for f in rem:
                        gen_f(f, f + 1)
                        for s in sign_fns[:per]:
                            s()
                        sign_fns = sign_fns[per:]
                    for s in sign_fns:
                        s()

                if len(chunk_bounds) == 1:
                    (f0, f1) = chunk_bounds[0]
                    gen_f(f0, f1, mid=pending, wave=True,
                          extra_dmas=extra_dmas)
                    ar = chunk_partA(0, f0, f1)
                    head, sign_fns = make_partB(0, f0, f1, ar)

                    def pend():
                        head()
                        for s in sign_fns:
                            s()

                    return out_segs, pend

                assert len(chunk_bounds) == 3, "expect 3 chunks at full size"
                (af0, af1), (bf0, bf1), (cf0, cf1) = chunk_bounds
                gen_f(af0, af1, mid=pending, wave=True, extra_dmas=extra_dmas)
                chunk_close(0, af0, af1, bf1)
                chunk_close(1, bf0, bf1, cf1)
                ar3 = chunk_partA(2, cf0, cf1)
                head3, signs3 = make_partB(2, cf0, cf1, ar3)

                def pend():
                    head3()
                    for s in signs3:
                        s()

                return out_segs, pend

            if not skip_collective:
                # Tiny rendezvous collective while the PE is still waiting on
                # the initial DMAs: absorbs cross-core start skew so layer
                # 1's real stats AllReduces see aligned cores (unaligned
                # first-ARs measured 3x slower).
                warm_in = dram.tile([P, 1], f32, tag="warm_in")
                warm_out = dram.tile([P, 1], f32, tag="warm_out")
                warm_sb = consts.tile([P, 1], f32)
                nc.vector.memset(warm_sb[:], 1.0)
                nc.gpsimd.dma_start(warm_in[:], warm_sb[:])
                nc.gpsimd.collective_compute(
                    "AllReduce",
                    Alu.add,
                    replica_groups=[list(range(n_cores))],
                    ins=[warm_in.opt()],
                    outs=[warm_out.opt()],
                )

            for _ in range(iters):
                a0 = acts.tile([P, ko_n, b_c], fp8, tag="act0")  # shares slots
                # with each layer's chunk-0 output (a0 is dead once layer 1's
                # matmuls finish, exactly when layer 2's chunk-0 allocates)
                # nb=0 half first so the first matmul group is gated only by
                # the first half + first weight tile
                nc.sync.dma_start(a0[:, :, 0:n_free], a0_d.ap()[:, :, 0:n_free])

                def a0_rest():
                    for nb in range(1, nb_n):
                        nc.sync.dma_start(
                            a0[:, :, ts(nb, n_free)],
                            a0_d.ap()[:, :, ts(nb, n_free)],
                        )

                segs, pend = hidden_layer(0, [(a0, 0, ko_n)],
                                          extra_dmas=a0_rest)
                segs, pend = hidden_layer(1, segs, pending=pend)
                segs, pend = hidden_layer(2, segs, pending=pend)

                # linear head: out = sign(a3) @ sign(Wout).T  (transposed)
                def head_sink(f, nb, ps):
                    o_sb = scratch.tile([P, n_free], f32, tag="o_sb")
                    nc.vector.tensor_copy(o_sb[:], ps)
                    nc.sync.dma_start(
                        out_d.ap()[:, f, ds(nb * n_free, n_free)], o_sb[:]
                    )

                gen_matmuls(wo_d.ap(), list(range(fo_n)), segs, head_sink,
                            mid=pend, wave=True)

    nc.compile()
    return nc


def _pack_weight(w_sign8: np.ndarray) -> np.ndarray:
    """[F, D] fp8 sign matrix -> lhsT tiles [F/P, P(d_part), D/P, P(m)]."""
    f, d = w_sign8.shape
    return np.ascontiguousarray(
        w_sign8.reshape(f // P, P, d // P, P).transpose(0, 3, 2, 1)
    )


def _pack_acts(x_sign8: np.ndarray) -> np.ndarray:
    """[B, D] fp8 sign matrix -> rhs tiles [P(d_part), D/P, B]."""
    b, d = x_sign8.shape
    return np.ascontiguousarray(x_sign8.T.reshape(d // P, P, b).transpose(1, 0, 2))


def _prep_inputs(x, W1, g1, b1, W2, g2, b2, W3, g3, b3, Wout, b_c, c_pad):
    xs = np.sign(x).astype(FP8)
    a0 = _pack_acts(xs)  # [P, D/P, B]
    ws = [_pack_weight(np.sign(w).astype(FP8)) for w in (W1, W2, W3)]
    c, hdim = Wout.shape
    wo8 = np.zeros((c_pad, hdim), FP8)
    wo8[:c] = np.sign(Wout).astype(FP8)
    wo = _pack_weight(wo8)
    gb = np.stack([g1, b1, g2, b2, g3, b3]).astype(np.float32)  # [6, H]
    hdim2 = gb.shape[1]
    gb = np.ascontiguousarray(gb.reshape(6, hdim2 // P, P).transpose(0, 2, 1))

    n_cores = a0.shape[2] // b_c
    in_maps = [
        {
            "a0": np.ascontiguousarray(a0[:, :, i * b_c : (i + 1) * b_c]),
            "w1": ws[0],
            "w2": ws[1],
            "w3": ws[2],
            "wo": wo,
            "gb": gb,
        }
        for i in range(n_cores)
    ]
    return in_maps


def _assemble_output(results, b_c, c: int) -> np.ndarray:
    """Per-core outT [P, c_pad/P, b_c] -> full [B, C] fp32."""
    blocks = []
    for r in results:
        o = r["outT"]  # [P, fo_n, b_c]
        blocks.append(o.transpose(1, 0, 2).reshape(-1, b_c))  # [c_pad, b_c]
    full = np.concatenate(blocks, axis=1)  # [c_pad, B]
    return np.ascontiguousarray(full[:c].T)


_NC_CACHE: dict = {}


def _get_nc(key, **kw):
    if key not in _NC_CACHE:
        _NC_CACHE[key] = build_kernel(**kw)
    return _NC_CACHE[key]


def kernel(x, W1, g1, b1, W2, g2, b2, W3, g3, b3, Wout):
    from concourse.bass_utils import run_bass_kernel_spmd

    b, d = x.shape
    hdim = W1.shape[0]
    c = Wout.shape[0]
    b_c = b // N_CORES
    c_pad = (c + P - 1) // P * P

    nc = _get_nc(
        (b_c, d, hdim, c_pad), b_c=b_c, d=d, h=hdim, c_pad=c_pad, n_cores=N_CORES
    )
    in_maps = _prep_inputs(
        x, W1, g1, b1, W2, g2, b2, W3, g3, b3, Wout, b_c, c_pad
    )
    res = run_bass_kernel_spmd(nc, in_maps, core_ids=list(range(N_CORES)))
    return _assemble_output(res.results, b_c, c)


def _np_reference(x, W1, g1, b1, W2, g2, b2, W3, g3, b3, Wout):
    """Inline numpy oracle for self-testing (mirrors the jax reference)."""
    h = np.sign(x).astype(np.float64)
    for W, g, bb in ((W1, g1, b1), (W2, g2, b2), (W3, g3, b3)):
        h = h @ np.sign(W).T.astype(np.float64)
        mean = h.mean(axis=0)
        var = h.var(axis=0)
        h = g * (h - mean) / np.sqrt(var + BN_EPS) + bb
        h = np.sign(h)
    return (h @ np.sign(Wout).T.astype(np.float64)).astype(np.float32)


def _selftest():
    rng = np.random.default_rng(1)
    b, d, hdim, c = 4096, 512, 512, 192
    x = rng.standard_normal((b, d)).astype(np.float32)
    W1 = (rng.standard_normal((hdim, d)) * 0.02).astype(np.float32)
    W2 = (rng.standard_normal((hdim, hdim)) * 0.02).astype(np.float32)
    W3 = (rng.standard_normal((hdim, hdim)) * 0.02).astype(np.float32)
    Wout = (rng.standard_normal((c, hdim)) * 0.02).astype(np.float32)
    g = np.ones(hdim, np.float32)
    bb = np.zeros(hdim, np.float32)

    got = kernel(x, W1, g, bb, W2, g, bb, W3, g, bb, Wout)
    want = _np_reference(x, W1, g, bb, W2, g, bb, W3, g, bb, Wout)
    diff = np.abs(got - want)
    denom = max(1e-9, np.abs(want).max())
    print(f"selftest: shape={got.shape} max_abs_err={diff.max()} "
          f"rel={diff.max() / denom:.3e} mismatches={(diff != 0).sum()}")


if __name__ == "__main__":
    _selftest()


# revision 33
# speedup vs baseline: 1.0699x; 1.0699x over previous
"""Trainium2 Bass kernel for nn_BinaryMLP (binary MLP with BatchNorm, 3 hidden
layers + linear head), distributed data-parallel over 8 NeuronCores.

Math per hidden layer (reference):
    h = sign(a_prev) @ sign(W).T          # [B, H], exact integers in fp32
    h = g * (h - mean) / sqrt(var + eps) + b   # batch stats over FULL batch
    a = sign(h)

Since sign() only cares about the side of a per-feature affine threshold,
BN+sign folds into  a = Sign(g * h + (b*sqrt(var+eps) - g*mean))  computed on
the ACT engine with per-partition (per-feature) scale/bias. All matmul
operands are in {-1, 0, +1}  ->  fp8e4 operands with fp32 PSUM accumulation
are EXACT. Batch mean = (exact integer sum) / 8192 is exact in fp32, so the
whole network is bit-exact vs the fp32 jax reference.

Sharding: batch 8192 -> 1024 rows per core, weights replicated. Activations
live on-chip transposed as [feature(part), batch(free)] so BN stats are
free-axis reductions and the per-feature threshold is a per-partition scalar.
Cross-core BN stats via a 32KB DRAM AllReduce per layer.
"""

import numpy as np
import ml_dtypes

P = 128
N_CORES = 8
BN_EPS = 1e-5

FP8 = ml_dtypes.float8_e4m3


def build_kernel(
    b_c: int,  # batch rows per core
    d: int,  # input features (= contraction dim of layer 1)
    h: int,  # hidden features
    c_pad: int,  # output features padded to a multiple of 128
    n_cores: int = N_CORES,
    iters: int = 1,
    n_free: int = 512,  # matmul moving free dim (PSUM bank)
    skip_collective: bool = False,  # timing experiments only (wrong results)
    chunks: tuple = (16, 28),  # feature-tile boundaries for stats/AR/sign
    psum_bufs: int = 8,
    w_bufs: int = 5,
    interleave_nb: bool = False,  # share stationary weights across nb blocks
):
    """Build + compile the SPMD Bass kernel. Returns the compiled Bacc."""
    import concourse.bass as bass
    import concourse.mybir as mybir
    import concourse.tile as tile
    from concourse import bacc
    from concourse.bass import ds, ts

    f32 = mybir.dt.float32
    fp16 = mybir.dt.float16
    fp8 = mybir.dt.float8e4
    Act = mybir.ActivationFunctionType
    Alu = mybir.AluOpType

    ko_n = d // P  # k-tiles layer 1
    kh_n = h // P  # k-tiles layers 2/3 and head
    f_n = h // P  # hidden feature tiles
    fo_n = c_pad // P  # head feature tiles
    nb_n = b_c // n_free  # batch blocks per core
    assert d % P == 0 and h % P == 0 and c_pad % P == 0 and b_c % n_free == 0
    inv_b = 1.0 / (b_c * n_cores)  # power of two -> exact fp32 scaling

    nc = bacc.Bacc(
        "TRN2", target_bir_lowering=False, debug=False, num_devices=n_cores
    )

    a0_d = nc.dram_tensor("a0", [P, ko_n, b_c], fp8, kind="ExternalInput")
    w_d = [
        nc.dram_tensor(f"w{l + 1}", [f_n, P, (ko_n if l == 0 else kh_n), P], fp8,
                       kind="ExternalInput")
        for l in range(3)
    ]
    wo_d = nc.dram_tensor("wo", [fo_n, P, kh_n, P], fp8, kind="ExternalInput")
    # (g1,b1,g2,b2,g3,b3) packed [6, P, f_n]
    gb_d = nc.dram_tensor("gb", [6, P, f_n], f32, kind="ExternalInput")
    out_d = nc.dram_tensor("outT", [P, fo_n, b_c], f32, kind="ExternalOutput")

    with tile.TileContext(nc) as tc:
        with (
            tc.tile_pool(name="acts", bufs=2) as acts,  # fp8 activations (ping/pong)
            tc.tile_pool(name="hbuf", bufs=1) as hbuf,  # fp16 pre-BN values
            tc.tile_pool(name="wpool", bufs=w_bufs) as wpool,
            tc.tile_pool(name="psum", bufs=psum_bufs, space="PSUM") as psum,
            tc.tile_pool(name="stats", bufs=4) as stats,
            tc.tile_pool(name="scratch", bufs=2) as scratch,
            tc.tile_pool(name="consts", bufs=1) as consts,
            tc.tile_pool(name="dram", bufs=4, space="DRAM") as dram,
        ):
            gb_sb = consts.tile([P, 6, f_n], f32)
            nc.sync.dma_start(gb_sb[:], gb_d.ap().rearrange("l p o -> p l o"))
            eps_t = consts.tile([P, 1], f32)
            nc.vector.memset(eps_t[:], BN_EPS)

            chunk_bounds = []
            f_start = 0
            for f_end in [c for c in chunks if 0 < c < f_n] + [f_n]:
                chunk_bounds.append((f_start, f_end))
                f_start = f_end

            def emit_pairs(ps, w_tile, segs, nb, seg_lo, seg_hi, first, last):
                """Emit DoubleRow matmul pairs for activation segments
                [seg_lo, seg_hi). segs: list of (tile, ko_off, ko_len)."""
                n_pairs = sum(kl for _, _, kl in segs[seg_lo:seg_hi]) // 2
                j = 0
                for a_seg, ko_off, ko_len in segs[seg_lo:seg_hi]:
                    for k2 in range(ko_len // 2):
                        nc.tensor.matmul(
                            ps,
                            lhsT=w_tile[:, ds(ko_off + 2 * k2, 2), :],
                            rhs=a_seg[:, ts(k2, 2), ds(nb * n_free, n_free)],
                            start=(first and j == 0),
                            stop=(last and j == n_pairs - 1),
                            perf_mode=mybir.MatmulPerfMode.DoubleRow,
                        )
                        j += 1

            def gen_matmuls(w_dram_t, f_list, segs, sink, mid=None,
                            wave=False, extra_dmas=None):
                """Emit matmul groups for feature tiles in f_list contracting
                over activation segments `segs`. With wave=True the first 8
                psum groups are emitted as a wave: all-but-last-segment
                partial products first, then `mid()` (the previous chunk's
                deferred AllReduce-readback + Sign work, which produces the
                last segment), then the last-segment products. Emission order
                tracks data-readiness order so Tile's static per-engine
                schedule never traps ready work behind blocked work."""
                wave_f = f_list[: 8 // nb_n] if (wave and len(segs) > 1) else []
                rest_f = [f for f in f_list if f not in wave_f]
                k_n = sum(kl for _, _, kl in segs)

                wtiles, pss = {}, {}
                for i, f in enumerate(wave_f):
                    w_tile = wpool.tile([P, k_n, P], fp8, tag="w")
                    nc.sync.dma_start(w_tile[:], w_dram_t[f])
                    if i == 0 and extra_dmas:
                        extra_dmas()
                        extra_dmas = None
                    wtiles[f] = w_tile
                    for nb in range(nb_n):
                        pss[(f, nb)] = psum.tile(
                            [P, n_free], f32, tag="ps", name=f"ps_w{f}_{nb}"
                        )
                for f in wave_f:
                    for nb in range(nb_n):
                        emit_pairs(pss[(f, nb)], wtiles[f], segs, nb,
                                   0, len(segs) - 1, first=True, last=False)
                if mid is not None:
                    mid()
                    mid = None
                for f in wave_f:
                    for nb in range(nb_n):
                        emit_pairs(pss[(f, nb)], wtiles[f], segs, nb,
                                   len(segs) - 1, len(segs), first=False,
                                   last=True)
                        sink(f, nb, pss[(f, nb)])
                for i, f in enumerate(rest_f):
                    w_tile = wpool.tile([P, k_n, P], fp8, tag="w")
                    nc.sync.dma_start(w_tile[:], w_dram_t[f])
                    if i == 0 and extra_dmas:
                        extra_dmas()
                        extra_dmas = None
                    if i == 0 and mid is not None:
                        mid()
                        mid = None
                    if interleave_nb:
                        # nb-interleaved: consecutive matmuls share the same
                        # stationary weight slice (codegen may skip reloads)
                        pss2 = [
                            psum.tile([P, n_free], f32, tag="ps",
                                      name=f"ps_i{f}_{nb}")
                            for nb in range(nb_n)
                        ]
                        n_pairs = k_n // 2
                        j = 0
                        for a_seg, ko_off, ko_len in segs:
                            for k2 in range(ko_len // 2):
                                for nb in range(nb_n):
                                    nc.tensor.matmul(
                                        pss2[nb],
                                        lhsT=w_tile[:, ds(ko_off + 2 * k2, 2), :],
                                        rhs=a_seg[:, ts(k2, 2),
                                                  ds(nb * n_free, n_free)],
                                        start=(j == 0),
                                        stop=(j == n_pairs - 1),
                                        perf_mode=mybir.MatmulPerfMode.DoubleRow,
                                    )
                                j += 1
                        for nb in range(nb_n):
                            sink(f, nb, pss2[nb])
                    else:
                        for nb in range(nb_n):
                            ps = psum.tile([P, n_free], f32, tag="ps")
                            emit_pairs(ps, w_tile, segs, nb, 0, len(segs),
                                       first=True, last=True)
                            sink(f, nb, ps)
                if mid is not None:
                    mid()

            def hidden_layer(l, segs, pending=None, extra_dmas=None):
                """Returns (out_segs, pending). Each chunk's epilogue is split
                into partA (stats reduce + AllReduce trigger, emitted right
                after the chunk's matmuls) and partB (readback + thresholds +
                Signs, emitted later — interleaved into subsequent matmul
                emission so every engine's static order matches readiness
                order). The final chunk's partB is returned as `pending` and
                is emitted inside the NEXT layer's first matmul wave."""
                h_sb = hbuf.tile([P, f_n, b_c], fp16, tag="h")
                g_ap = gb_sb[:, 2 * l, :]
                b_ap = gb_sb[:, 2 * l + 1, :]
                out_segs = []
                accs = {}

                def chunk_partA(ci, f0, f1):
                    csz = f1 - f0
                    sum_acc, sq_acc = accs[ci]
                    # local stats -> AllReduce (trigger only)
                    stat_sb = stats.tile([P, 2 * csz], f32, tag="stat_sb")
                    nc.vector.tensor_reduce(
                        stat_sb[:, 0:csz], sum_acc[:, f0:f1, :],
                        mybir.AxisListType.X, Alu.add,
                    )
                    nc.vector.tensor_reduce(
                        stat_sb[:, csz : 2 * csz], sq_acc[:, f0:f1, :],
                        mybir.AxisListType.X, Alu.add,
                    )
                    if skip_collective:
                        return stat_sb
                    cc_in = dram.tile([P, 2 * csz], f32, tag="cc_in")
                    cc_out = dram.tile([P, 2 * csz], f32, tag="cc_out")
                    nc.gpsimd.dma_start(cc_in[:], stat_sb[:])
                    nc.gpsimd.collective_compute(
                        "AllReduce",
                        Alu.add,
                        replica_groups=[list(range(n_cores))],
                        ins=[cc_in.opt()],
                        outs=[cc_out.opt()],
                    )
                    return cc_out

                def make_partB(ci, f0, f1, ar_out):
                    """Returns (head, [sign_fn...]): head does the AllReduce
                    readback + threshold math; each sign_fn emits one feature
                    tile's Sign. Emitted piecemeal between later feature
                    tiles so ACT never has a long blocked burst queued ahead
                    of PSUM-recycling copies."""
                    csz = f1 - f0
                    a_out = acts.tile([P, csz, b_c], fp8, tag=f"act{ci}")
                    out_segs.append((a_out, f0, csz))
                    cvec = stats.tile([P, csz], f32, tag=f"cvec{ci}",
                                      name=f"cvec_{l}_{ci}")

                    def head():
                        if skip_collective:
                            gstat = ar_out
                        else:
                            gstat = stats.tile([P, 2 * csz], f32, tag="gstat")
                            nc.sync.dma_start(gstat[:], ar_out[:])
                        # threshold: a = Sign(g*h + (b*std - g*mean))
                        mean_t = stats.tile([P, csz], f32, tag="mean_t")
                        var_t = stats.tile([P, csz], f32, tag="var_t")
                        std_t = stats.tile([P, csz], f32, tag="std_t")
                        tmp_t = stats.tile([P, csz], f32, tag="tmp_t")
                        nc.vector.tensor_scalar_mul(
                            mean_t[:], gstat[:, 0:csz], inv_b
                        )
                        nc.vector.tensor_scalar_mul(
                            tmp_t[:], gstat[:, csz : 2 * csz], inv_b
                        )
                        nc.vector.tensor_tensor(
                            var_t[:], mean_t[:], mean_t[:], Alu.mult
                        )
                        nc.vector.tensor_tensor(
                            var_t[:], tmp_t[:], var_t[:], Alu.subtract
                        )
                        nc.scalar.activation(
                            std_t[:], var_t[:], Act.Sqrt, bias=eps_t[:]
                        )
                        nc.vector.tensor_tensor(
                            tmp_t[:], b_ap[:, f0:f1], std_t[:], Alu.mult
                        )
                        nc.vector.tensor_tensor(
                            std_t[:], g_ap[:, f0:f1], mean_t[:], Alu.mult
                        )
                        nc.vector.tensor_tensor(
                            cvec[:], tmp_t[:], std_t[:], Alu.subtract
                        )

                    def sign_of(f):
                        def emit():
                            nc.scalar.activation(
                                a_out[:, f - f0, :], h_sb[:, f, :], Act.Sign,
                                bias=cvec[:, f - f0 : f - f0 + 1],
                                scale=g_ap[:, f : f + 1],
                            )
                        return emit

                    return head, [sign_of(f) for f in range(f0, f1)]

                def sink(f, nb, ps):
                    ci = next(i for i, (lo, hi) in enumerate(chunk_bounds)
                              if lo <= f < hi)
                    sum_acc, sq_acc = accs[ci]
                    # ACT: copy to fp16 h (exact) + per-feature batch sum
                    nc.scalar.activation(
                        h_sb[:, f, ts(nb, n_free)], ps, Act.Copy,
                        accum_out=sum_acc[:, f, nb : nb + 1],
                    )
                    # DVE: square from the fp16 copy, then sum
                    # (only one PSUM operand allowed per DVE op)
                    hh = h_sb[:, f, ts(nb, n_free)]
                    sq_scr = scratch.tile([P, n_free], f32, tag="sq_scr")
                    nc.vector.tensor_tensor(sq_scr[:], hh, hh, Alu.mult)
                    nc.vector.tensor_reduce(
                        sq_acc[:, f, nb : nb + 1], sq_scr[:],
                        mybir.AxisListType.X, Alu.add,
                    )

                for ci in range(len(chunk_bounds)):
                    accs[ci] = (
                        stats.tile([P, f_n, nb_n], f32, tag="sum_acc",
                                   name=f"sum_acc_{l}_{ci}"),
                        stats.tile([P, f_n, nb_n], f32, tag="sq_acc",
                                   name=f"sq_acc_{l}_{ci}"),
                    )

                w_ap = w_d[l].ap()

                def gen_f(f_lo, f_hi, **kw):
                    gen_matmuls(w_ap, list(range(f_lo, f_hi)), segs, sink,
                                **kw)

                def chunk_close(ci, f0, f1, cover_hi):
                    """partA for chunk ci, then its partB spread over feature
                    tiles [f1, cover_hi): 2 ftiles of matmul cover while the
                    AllReduce flies, then Signs trickled between the rest."""
                    ar = chunk_partA(ci, f0, f1)
                    gen_f(f1, min(f1 + 2, cover_hi))
                    head, sign_fns = make_partB(ci, f0, f1, ar)
                    head()
                    rem = list(range(min(f1 + 2, cover_hi), cover_hi))
                    per = -(-len(sign_fns) // max(1, len(rem)))
                    for f in rem:
                        gen_f(f, f + 1)
                        for s in sign_fns[:per]:
                            s()
                        sign_fns = sign_fns[per:]
                    for s in sign_fns:
                        s()

                if len(chunk_bounds) == 1:
                    (f0, f1) = chunk_bounds[0]
                    gen_f(f0, f1, mid=pending, wave=True,
                          extra_dmas=extra_dmas)
                    ar = chunk_partA(0, f0, f1)
                    head, sign_fns = make_partB(0, f0, f1, ar)

                    def pend():
                        head()
                        for s in sign_fns:
                            s()

                    return out_segs, pend

                assert len(chunk_bounds) == 3, "expect 3 chunks at full size"
                (af0, af1), (bf0, bf1), (cf0, cf1) = chunk_bounds
                gen_f(af0, af1, mid=pending, wave=True, extra_dmas=extra_dmas)
                chunk_close(0, af0, af1, bf1)
                chunk_close(1, bf0, bf1, cf1)
                ar3 = chunk_partA(2, cf0, cf1)
                head3, signs3 = make_partB(2, cf0, cf1, ar3)

                def pend():
                    head3()
                    for s in signs3:
                        s()

                return out_segs, pend

            if not skip_collective:
                # Tiny rendezvous collective while the PE is still waiting on
                # the initial DMAs: absorbs cross-core start skew so layer
                # 1's real stats AllReduces see aligned cores (unaligned
                # first-ARs measured 3x slower).
                warm_in = dram.tile([P, 1], f32, tag="warm_in")
                warm_out = dram.tile([P, 1], f32, tag="warm_out")
                warm_sb = consts.tile([P, 1], f32)
                nc.vector.memset(warm_sb[:], 1.0)
                nc.gpsimd.dma_start(warm_in[:], warm_sb[:])
                nc.gpsimd.collective_compute(
                    "AllReduce",
                    Alu.add,
                    replica_groups=[list(range(n_cores))],
                    ins=[warm_in.opt()],
                    outs=[warm_out.opt()],
                )

            for _ in range(iters):
                # a0 split into two k-segments (tags shared with the layer
                # chunk outputs; dead once layer 1's matmuls finish). The
                # first wave then needs only a0a's first batch half + one
                # weight tile (~1.5MB) instead of all of a0 (~4.7MB), cutting
                # the startup DMA ramp on the critical path.
                k_half = ko_n // 2
                a0a = acts.tile([P, k_half, b_c], fp8, tag="act0")
                a0b = acts.tile([P, ko_n - k_half, b_c], fp8, tag="act1")
                nc.sync.dma_start(
                    a0a[:, :, 0:n_free], a0_d.ap()[:, 0:k_half, 0:n_free]
                )

                def a0_rest():
                    for nb in range(1, nb_n):
                        nc.sync.dma_start(
                            a0a[:, :, ts(nb, n_free)],
                            a0_d.ap()[:, 0:k_half, ts(nb, n_free)],
                        )
                    for nb in range(nb_n):
                        nc.sync.dma_start(
                            a0b[:, :, ts(nb, n_free)],
                            a0_d.ap()[:, k_half:ko_n, ts(nb, n_free)],
                        )

                segs, pend = hidden_layer(
                    0, [(a0a, 0, k_half), (a0b, k_half, ko_n - k_half)],
                    extra_dmas=a0_rest,
                )
                segs, pend = hidden_layer(1, segs, pending=pend)
                segs, pend = hidden_layer(2, segs, pending=pend)

                # linear head: out = sign(a3) @ sign(Wout).T  (transposed)
                def head_sink(f, nb, ps):
                    o_sb = scratch.tile([P, n_free], f32, tag="o_sb")
                    nc.vector.tensor_copy(o_sb[:], ps)
                    nc.sync.dma_start(
                        out_d.ap()[:, f, ds(nb * n_free, n_free)], o_sb[:]
                    )

                gen_matmuls(wo_d.ap(), list(range(fo_n)), segs, head_sink,
                            mid=pend, wave=True)

    nc.compile()
    return nc


def _pack_weight(w_sign8: np.ndarray) -> np.ndarray:
    """[F, D] fp8 sign matrix -> lhsT tiles [F/P, P(d_part), D/P, P(m)]."""
    f, d = w_sign8.shape
    return np.ascontiguousarray(
        w_sign8.reshape(f // P, P, d // P, P).transpose(0, 3, 2, 1)
    )


def _pack_acts(x_sign8: np.ndarray) -> np.ndarray:
    """[B, D] fp8 sign matrix -> rhs tiles [P(d_part), D/P, B]."""
    b, d = x_sign8.shape
    return np.ascontiguousarray(x_sign8.T.reshape(d // P, P, b).transpose(1, 0, 2))


def _prep_inputs(x, W1, g1, b1, W2, g2, b2, W3, g3, b3, Wout, b_c, c_pad):
    xs = np.sign(x).astype(FP8)
    a0 = _pack_acts(xs)  # [P, D/P, B]
    ws = [_pack_weight(np.sign(w).astype(FP8)) for w in (W1, W2, W3)]
    c, hdim = Wout.shape
    wo8 = np.zeros((c_pad, hdim), FP8)
    wo8[:c] = np.sign(Wout).astype(FP8)
    wo = _pack_weight(wo8)
    gb = np.stack([g1, b1, g2, b2, g3, b3]).astype(np.float32)  # [6, H]
    hdim2 = gb.shape[1]
    gb = np.ascontiguousarray(gb.reshape(6, hdim2 // P, P).transpose(0, 2, 1))

    n_cores = a0.shape[2] // b_c
    in_maps = [
        {
            "a0": np.ascontiguousarray(a0[:, :, i * b_c : (i + 1) * b_c]),
            "w1": ws[0],
            "w2": ws[1],
            "w3": ws[2],
            "wo": wo,
            "gb": gb,
        }
        for i in range(n_cores)
    ]
    return in_maps


def _assemble_output(results, b_c, c: int) -> np.ndarray:
    """Per-core outT [P, c_pad/P, b_c] -> full [B, C] fp32."""
    blocks = []
    for r in results:
        o = r["outT"]  # [P, fo_n, b_c]
        blocks.append(o.transpose(1, 0, 2).reshape(-1, b_c))  # [c_pad, b_c]
    full = np.concatenate(blocks, axis=1)  # [c_pad, B]
    return np.ascontiguousarray(full[:c].T)


_NC_CACHE: dict = {}


def _get_nc(key, **kw):
    if key not in _NC_CACHE:
        _NC_CACHE[key] = build_kernel(**kw)
    return _NC_CACHE[key]


def kernel(x, W1, g1, b1, W2, g2, b2, W3, g3, b3, Wout):
    from concourse.bass_utils import run_bass_kernel_spmd

    b, d = x.shape
    hdim = W1.shape[0]
    c = Wout.shape[0]
    b_c = b // N_CORES
    c_pad = (c + P - 1) // P * P

    nc = _get_nc(
        (b_c, d, hdim, c_pad), b_c=b_c, d=d, h=hdim, c_pad=c_pad, n_cores=N_CORES
    )
    in_maps = _prep_inputs(
        x, W1, g1, b1, W2, g2, b2, W3, g3, b3, Wout, b_c, c_pad
    )
    res = run_bass_kernel_spmd(nc, in_maps, core_ids=list(range(N_CORES)))
    return _assemble_output(res.results, b_c, c)


def _np_reference(x, W1, g1, b1, W2, g2, b2, W3, g3, b3, Wout):
    """Inline numpy oracle for self-testing (mirrors the jax reference)."""
    h = np.sign(x).astype(np.float64)
    for W, g, bb in ((W1, g1, b1), (W2, g2, b2), (W3, g3, b3)):
        h = h @ np.sign(W).T.astype(np.float64)
        mean = h.mean(axis=0)
        var = h.var(axis=0)
        h = g * (h - mean) / np.sqrt(var + BN_EPS) + bb
        h = np.sign(h)
    return (h @ np.sign(Wout).T.astype(np.float64)).astype(np.float32)


def _selftest():
    rng = np.random.default_rng(1)
    b, d, hdim, c = 4096, 512, 512, 192
    x = rng.standard_normal((b, d)).astype(np.float32)
    W1 = (rng.standard_normal((hdim, d)) * 0.02).astype(np.float32)
    W2 = (rng.standard_normal((hdim, hdim)) * 0.02).astype(np.float32)
    W3 = (rng.standard_normal((hdim, hdim)) * 0.02).astype(np.float32)
    Wout = (rng.standard_normal((c, hdim)) * 0.02).astype(np.float32)
    g = np.ones(hdim, np.float32)
    bb = np.zeros(hdim, np.float32)

    got = kernel(x, W1, g, bb, W2, g, bb, W3, g, bb, Wout)
    want = _np_reference(x, W1, g, bb, W2, g, bb, W3, g, bb, Wout)
    diff = np.abs(got - want)
    denom = max(1e-9, np.abs(want).max())
    print(f"selftest: shape={got.shape} max_abs_err={diff.max()} "
          f"rel={diff.max() / denom:.3e} mismatches={(diff != 0).sum()}")


if __name__ == "__main__":
    _selftest()


# revision 35
# speedup vs baseline: 1.0809x; 1.0103x over previous
"""Trainium2 Bass kernel for nn_BinaryMLP (binary MLP with BatchNorm, 3 hidden
layers + linear head), distributed data-parallel over 8 NeuronCores.

Math per hidden layer (reference):
    h = sign(a_prev) @ sign(W).T          # [B, H], exact integers in fp32
    h = g * (h - mean) / sqrt(var + eps) + b   # batch stats over FULL batch
    a = sign(h)

Since sign() only cares about the side of a per-feature affine threshold,
BN+sign folds into  a = Sign(g * h + (b*sqrt(var+eps) - g*mean))  computed on
the ACT engine with per-partition (per-feature) scale/bias. All matmul
operands are in {-1, 0, +1}  ->  fp8e4 operands with fp32 PSUM accumulation
are EXACT. Batch mean = (exact integer sum) / 8192 is exact in fp32, so the
whole network is bit-exact vs the fp32 jax reference.

Sharding: batch 8192 -> 1024 rows per core, weights replicated. Activations
live on-chip transposed as [feature(part), batch(free)] so BN stats are
free-axis reductions and the per-feature threshold is a per-partition scalar.
Cross-core BN stats via a 32KB DRAM AllReduce per layer.
"""

import numpy as np
import ml_dtypes

P = 128
N_CORES = 8
BN_EPS = 1e-5

FP8 = ml_dtypes.float8_e4m3


def build_kernel(
    b_c: int,  # batch rows per core
    d: int,  # input features (= contraction dim of layer 1)
    h: int,  # hidden features
    c_pad: int,  # output features padded to a multiple of 128
    n_cores: int = N_CORES,
    iters: int = 1,
    n_free: int = 512,  # matmul moving free dim (PSUM bank)
    skip_collective: bool = False,  # timing experiments only (wrong results)
    chunks: tuple = (16, 28),  # feature-tile boundaries for stats/AR/sign
    psum_bufs: int = 8,
    w_bufs: int = 5,
    interleave_nb: bool = False,  # share stationary weights across nb blocks
):
    """Build + compile the SPMD Bass kernel. Returns the compiled Bacc."""
    import concourse.bass as bass
    import concourse.mybir as mybir
    import concourse.tile as tile
    from concourse import bacc
    from concourse.bass import ds, ts

    f32 = mybir.dt.float32
    fp16 = mybir.dt.float16
    fp8 = mybir.dt.float8e4
    Act = mybir.ActivationFunctionType
    Alu = mybir.AluOpType

    ko_n = d // P  # k-tiles layer 1
    kh_n = h // P  # k-tiles layers 2/3 and head
    f_n = h // P  # hidden feature tiles
    fo_n = c_pad // P  # head feature tiles
    nb_n = b_c // n_free  # batch blocks per core
    assert d % P == 0 and h % P == 0 and c_pad % P == 0 and b_c % n_free == 0
    inv_b = 1.0 / (b_c * n_cores)  # power of two -> exact fp32 scaling

    nc = bacc.Bacc(
        "TRN2", target_bir_lowering=False, debug=False, num_devices=n_cores
    )

    a0_d = nc.dram_tensor("a0", [P, ko_n, b_c], fp8, kind="ExternalInput")
    w_d = [
        nc.dram_tensor(f"w{l + 1}", [f_n, P, (ko_n if l == 0 else kh_n), P], fp8,
                       kind="ExternalInput")
        for l in range(3)
    ]
    wo_d = nc.dram_tensor("wo", [fo_n, P, kh_n, P], fp8, kind="ExternalInput")
    # (g1,b1,g2,b2,g3,b3) packed [6, P, f_n]
    gb_d = nc.dram_tensor("gb", [6, P, f_n], f32, kind="ExternalInput")
    out_d = nc.dram_tensor("outT", [P, fo_n, b_c], f32, kind="ExternalOutput")

    with tile.TileContext(nc) as tc:
        with (
            tc.tile_pool(name="acts", bufs=2) as acts,  # fp8 activations (ping/pong)
            tc.tile_pool(name="hbuf", bufs=1) as hbuf,  # fp16 pre-BN values
            tc.tile_pool(name="wpool", bufs=w_bufs) as wpool,
            tc.tile_pool(name="psum", bufs=psum_bufs, space="PSUM") as psum,
            tc.tile_pool(name="stats", bufs=4) as stats,
            tc.tile_pool(name="scratch", bufs=2) as scratch,
            tc.tile_pool(name="consts", bufs=1) as consts,
            tc.tile_pool(name="dram", bufs=4, space="DRAM") as dram,
        ):
            gb_sb = consts.tile([P, 6, f_n], f32)
            nc.sync.dma_start(gb_sb[:], gb_d.ap().rearrange("l p o -> p l o"))
            eps_t = consts.tile([P, 1], f32)
            nc.vector.memset(eps_t[:], BN_EPS)

            chunk_bounds = []
            f_start = 0
            for f_end in [c for c in chunks if 0 < c < f_n] + [f_n]:
                chunk_bounds.append((f_start, f_end))
                f_start = f_end

            def emit_pairs(ps, w_tile, segs, nb, seg_lo, seg_hi, first, last):
                """Emit DoubleRow matmul pairs for activation segments
                [seg_lo, seg_hi). segs: list of (tile, ko_off, ko_len)."""
                n_pairs = sum(kl for _, _, kl in segs[seg_lo:seg_hi]) // 2
                j = 0
                for a_seg, ko_off, ko_len in segs[seg_lo:seg_hi]:
                    for k2 in range(ko_len // 2):
                        nc.tensor.matmul(
                            ps,
                            lhsT=w_tile[:, ds(ko_off + 2 * k2, 2), :],
                            rhs=a_seg[:, ts(k2, 2), ds(nb * n_free, n_free)],
                            start=(first and j == 0),
                            stop=(last and j == n_pairs - 1),
                            perf_mode=mybir.MatmulPerfMode.DoubleRow,
                        )
                        j += 1

            def gen_matmuls(w_dram_t, f_list, segs, sink, mid=None,
                            wave=False, extra_dmas=None):
                """Emit matmul groups for feature tiles in f_list contracting
                over activation segments `segs`. With wave=True the first 8
                psum groups are emitted as a wave: all-but-last-segment
                partial products first, then `mid()` (the previous chunk's
                deferred AllReduce-readback + Sign work, which produces the
                last segment), then the last-segment products. Emission order
                tracks data-readiness order so Tile's static per-engine
                schedule never traps ready work behind blocked work."""
                wave_f = f_list[: 8 // nb_n] if (wave and len(segs) > 1) else []
                rest_f = [f for f in f_list if f not in wave_f]
                k_n = sum(kl for _, _, kl in segs)

                wtiles, pss = {}, {}
                for i, f in enumerate(wave_f):
                    w_tile = wpool.tile([P, k_n, P], fp8, tag="w")
                    nc.sync.dma_start(w_tile[:], w_dram_t[f])
                    if i == 0 and extra_dmas:
                        extra_dmas()
                        extra_dmas = None
                    wtiles[f] = w_tile
                    for nb in range(nb_n):
                        pss[(f, nb)] = psum.tile(
                            [P, n_free], f32, tag="ps", name=f"ps_w{f}_{nb}"
                        )
                for f in wave_f:
                    for nb in range(nb_n):
                        emit_pairs(pss[(f, nb)], wtiles[f], segs, nb,
                                   0, len(segs) - 1, first=True, last=False)
                if mid is not None:
                    mid()
                    mid = None
                for f in wave_f:
                    for nb in range(nb_n):
                        emit_pairs(pss[(f, nb)], wtiles[f], segs, nb,
                                   len(segs) - 1, len(segs), first=False,
                                   last=True)
                        sink(f, nb, pss[(f, nb)])
                for i, f in enumerate(rest_f):
                    w_tile = wpool.tile([P, k_n, P], fp8, tag="w")
                    nc.sync.dma_start(w_tile[:], w_dram_t[f])
                    if i == 0 and extra_dmas:
                        extra_dmas()
                        extra_dmas = None
                    if i == 0 and mid is not None:
                        mid()
                        mid = None
                    if interleave_nb:
                        # nb-interleaved: consecutive matmuls share the same
                        # stationary weight slice (codegen may skip reloads)
                        pss2 = [
                            psum.tile([P, n_free], f32, tag="ps",
                                      name=f"ps_i{f}_{nb}")
                            for nb in range(nb_n)
                        ]
                        n_pairs = k_n // 2
                        j = 0
                        for a_seg, ko_off, ko_len in segs:
                            for k2 in range(ko_len // 2):
                                for nb in range(nb_n):
                                    nc.tensor.matmul(
                                        pss2[nb],
                                        lhsT=w_tile[:, ds(ko_off + 2 * k2, 2), :],
                                        rhs=a_seg[:, ts(k2, 2),
                                                  ds(nb * n_free, n_free)],
                                        start=(j == 0),
                                        stop=(j == n_pairs - 1),
                                        perf_mode=mybir.MatmulPerfMode.DoubleRow,
                                    )
                                j += 1
                        for nb in range(nb_n):
                            sink(f, nb, pss2[nb])
                    else:
                        for nb in range(nb_n):
                            ps = psum.tile([P, n_free], f32, tag="ps")
                            emit_pairs(ps, w_tile, segs, nb, 0, len(segs),
                                       first=True, last=True)
                            sink(f, nb, ps)
                if mid is not None:
                    mid()

            def hidden_layer(l, segs, pending=None, extra_dmas=None):
                """Returns (out_segs, pending). Each chunk's epilogue is split
                into partA (stats reduce + AllReduce trigger, emitted right
                after the chunk's matmuls) and partB (readback + thresholds +
                Signs, emitted later — interleaved into subsequent matmul
                emission so every engine's static order matches readiness
                order). The final chunk's partB is returned as `pending` and
                is emitted inside the NEXT layer's first matmul wave."""
                h_sb = hbuf.tile([P, f_n, b_c], fp16, tag="h")
                g_ap = gb_sb[:, 2 * l, :]
                b_ap = gb_sb[:, 2 * l + 1, :]
                out_segs = []
                accs = {}

                def chunk_partA(ci, f0, f1):
                    csz = f1 - f0
                    sum_acc, sq_acc = accs[ci]
                    # local stats -> AllReduce (trigger only)
                    stat_sb = stats.tile([P, 2 * csz], f32, tag="stat_sb")
                    nc.vector.tensor_reduce(
                        stat_sb[:, 0:csz], sum_acc[:, f0:f1, :],
                        mybir.AxisListType.X, Alu.add,
                    )
                    nc.vector.tensor_reduce(
                        stat_sb[:, csz : 2 * csz], sq_acc[:, f0:f1, :],
                        mybir.AxisListType.X, Alu.add,
                    )
                    if skip_collective:
                        return stat_sb
                    cc_in = dram.tile([P, 2 * csz], f32, tag="cc_in")
                    cc_out = dram.tile([P, 2 * csz], f32, tag="cc_out")
                    nc.gpsimd.dma_start(cc_in[:], stat_sb[:])
                    nc.gpsimd.collective_compute(
                        "AllReduce",
                        Alu.add,
                        replica_groups=[list(range(n_cores))],
                        ins=[cc_in.opt()],
                        outs=[cc_out.opt()],
                    )
                    return cc_out

                def make_partB(ci, f0, f1, ar_out):
                    """Returns (head, [sign_fn...]): head does the AllReduce
                    readback + threshold math; each sign_fn emits one feature
                    tile's Sign. Emitted piecemeal between later feature
                    tiles so ACT never has a long blocked burst queued ahead
                    of PSUM-recycling copies."""
                    csz = f1 - f0
                    a_out = acts.tile([P, csz, b_c], fp8, tag=f"act{ci}")
                    out_segs.append((a_out, f0, csz))
                    cvec = stats.tile([P, csz], f32, tag=f"cvec{ci}",
                                      name=f"cvec_{l}_{ci}")

                    def head():
                        if skip_collective:
                            gstat = ar_out
                        else:
                            gstat = stats.tile([P, 2 * csz], f32, tag="gstat")
                            nc.sync.dma_start(gstat[:], ar_out[:])
                        # threshold: a = Sign(g*h + (b*std - g*mean))
                        mean_t = stats.tile([P, csz], f32, tag="mean_t")
                        var_t = stats.tile([P, csz], f32, tag="var_t")
                        std_t = stats.tile([P, csz], f32, tag="std_t")
                        tmp_t = stats.tile([P, csz], f32, tag="tmp_t")
                        nc.vector.tensor_scalar_mul(
                            mean_t[:], gstat[:, 0:csz], inv_b
                        )
                        nc.vector.tensor_scalar_mul(
                            tmp_t[:], gstat[:, csz : 2 * csz], inv_b
                        )
                        nc.vector.tensor_tensor(
                            var_t[:], mean_t[:], mean_t[:], Alu.mult
                        )
                        nc.vector.tensor_tensor(
                            var_t[:], tmp_t[:], var_t[:], Alu.subtract
                        )
                        nc.scalar.activation(
                            std_t[:], var_t[:], Act.Sqrt, bias=eps_t[:]
                        )
                        nc.vector.tensor_tensor(
                            tmp_t[:], b_ap[:, f0:f1], std_t[:], Alu.mult
                        )
                        nc.vector.tensor_tensor(
                            std_t[:], g_ap[:, f0:f1], mean_t[:], Alu.mult
                        )
                        nc.vector.tensor_tensor(
                            cvec[:], tmp_t[:], std_t[:], Alu.subtract
                        )

                    def sign_of(f):
                        def emit():
                            nc.scalar.activation(
                                a_out[:, f - f0, :], h_sb[:, f, :], Act.Sign,
                                bias=cvec[:, f - f0 : f - f0 + 1],
                                scale=g_ap[:, f : f + 1],
                            )
                        return emit

                    return head, [sign_of(f) for f in range(f0, f1)]

                def sink(f, nb, ps):
                    ci = next(i for i, (lo, hi) in enumerate(chunk_bounds)
                              if lo <= f < hi)
                    sum_acc, sq_acc = accs[ci]
                    # ACT: copy to fp16 h (exact) + per-feature batch sum
                    nc.scalar.activation(
                        h_sb[:, f, ts(nb, n_free)], ps, Act.Copy,
                        accum_out=sum_acc[:, f, nb : nb + 1],
                    )
                    # DVE: square from the fp16 copy, then sum
                    # (only one PSUM operand allowed per DVE op)
                    hh = h_sb[:, f, ts(nb, n_free)]
                    sq_scr = scratch.tile([P, n_free], f32, tag="sq_scr")
                    nc.vector.tensor_tensor(sq_scr[:], hh, hh, Alu.mult)
                    nc.vector.tensor_reduce(
                        sq_acc[:, f, nb : nb + 1], sq_scr[:],
                        mybir.AxisListType.X, Alu.add,
                    )

                for ci in range(len(chunk_bounds)):
                    accs[ci] = (
                        stats.tile([P, f_n, nb_n], f32, tag="sum_acc",
                                   name=f"sum_acc_{l}_{ci}"),
                        stats.tile([P, f_n, nb_n], f32, tag="sq_acc",
                                   name=f"sq_acc_{l}_{ci}"),
                    )

                w_ap = w_d[l].ap()

                def gen_f(f_lo, f_hi, **kw):
                    gen_matmuls(w_ap, list(range(f_lo, f_hi)), segs, sink,
                                **kw)

                def chunk_close(ci, f0, f1, cover_hi):
                    """partA for chunk ci, then its partB spread over feature
                    tiles [f1, cover_hi): 2 ftiles of matmul cover while the
                    AllReduce flies, then Signs trickled between the rest."""
                    ar = chunk_partA(ci, f0, f1)
                    gen_f(f1, min(f1 + 2, cover_hi))
                    head, sign_fns = make_partB(ci, f0, f1, ar)
                    head()
                    rem = list(range(min(f1 + 2, cover_hi), cover_hi))
                    per = -(-len(sign_fns) // max(1, len(rem)))
                    for f in rem:
                        gen_f(f, f + 1)
                        for s in sign_fns[:per]:
                            s()
                        sign_fns = sign_fns[per:]
                    for s in sign_fns:
                        s()

                if len(chunk_bounds) == 1:
                    (f0, f1) = chunk_bounds[0]
                    gen_f(f0, f1, mid=pending, wave=True,
                          extra_dmas=extra_dmas)
                    ar = chunk_partA(0, f0, f1)
                    head, sign_fns = make_partB(0, f0, f1, ar)

                    def pend():
                        head()
                        for s in sign_fns:
                            s()

                    return out_segs, pend

                assert len(chunk_bounds) == 3, "expect 3 chunks at full size"
                (af0, af1), (bf0, bf1), (cf0, cf1) = chunk_bounds
                gen_f(af0, af1, mid=pending, wave=True, extra_dmas=extra_dmas)
                chunk_close(0, af0, af1, bf1)
                chunk_close(1, bf0, bf1, cf1)
                ar3 = chunk_partA(2, cf0, cf1)
                head3, signs3 = make_partB(2, cf0, cf1, ar3)

                def pend():
                    head3()
                    for s in signs3:
                        s()

                return out_segs, pend

            if not skip_collective:
                # Tiny rendezvous collective while the PE is still waiting on
                # the initial DMAs: absorbs cross-core start skew so layer
                # 1's real stats AllReduces see aligned cores (unaligned
                # first-ARs measured 3x slower).
                warm_in = dram.tile([P, 1], f32, tag="warm_in")
                warm_out = dram.tile([P, 1], f32, tag="warm_out")
                warm_sb = consts.tile([P, 1], f32)
                nc.vector.memset(warm_sb[:], 1.0)
                nc.gpsimd.dma_start(warm_in[:], warm_sb[:])
                nc.gpsimd.collective_compute(
                    "AllReduce",
                    Alu.add,
                    replica_groups=[list(range(n_cores))],
                    ins=[warm_in.opt()],
                    outs=[warm_out.opt()],
                )

            for _ in range(iters):
                # a0 split into two k-segments (tags shared with the layer
                # chunk outputs; dead once layer 1's matmuls finish). The
                # first wave then needs only a0a's first batch half + one
                # weight tile (~1.5MB) instead of all of a0 (~4.7MB), cutting
                # the startup DMA ramp on the critical path.
                k_half = ko_n // 2
                a0a = acts.tile([P, k_half, b_c], fp8, tag="act0")
                a0b = acts.tile([P, ko_n - k_half, b_c], fp8, tag="act1")
                nc.sync.dma_start(
                    a0a[:, :, 0:n_free], a0_d.ap()[:, 0:k_half, 0:n_free]
                )

                def a0_rest():
                    # issue from the scalar engine: lands in a different DMA
                    # queue group than the sync-engine weight stream, so the
                    # startup ramp uses two queue groups in parallel
                    for nb in range(1, nb_n):
                        nc.scalar.dma_start(
                            a0a[:, :, ts(nb, n_free)],
                            a0_d.ap()[:, 0:k_half, ts(nb, n_free)],
                        )
                    for nb in range(nb_n):
                        nc.scalar.dma_start(
                            a0b[:, :, ts(nb, n_free)],
                            a0_d.ap()[:, k_half:ko_n, ts(nb, n_free)],
                        )

                segs, pend = hidden_layer(
                    0, [(a0a, 0, k_half), (a0b, k_half, ko_n - k_half)],
                    extra_dmas=a0_rest,
                )
                segs, pend = hidden_layer(1, segs, pending=pend)
                segs, pend = hidden_layer(2, segs, pending=pend)

                # linear head: out = sign(a3) @ sign(Wout).T  (transposed)
                def head_sink(f, nb, ps):
                    o_sb = scratch.tile([P, n_free], f32, tag="o_sb")
                    nc.vector.tensor_copy(o_sb[:], ps)
                    nc.sync.dma_start(
                        out_d.ap()[:, f, ds(nb * n_free, n_free)], o_sb[:]
                    )

                gen_matmuls(wo_d.ap(), list(range(fo_n)), segs, head_sink,
                            mid=pend, wave=True)

    nc.compile()
    return nc


def _pack_weight(w_sign8: np.ndarray) -> np.ndarray:
    """[F, D] fp8 sign matrix -> lhsT tiles [F/P, P(d_part), D/P, P(m)]."""
    f, d = w_sign8.shape
    return np.ascontiguousarray(
        w_sign8.reshape(f // P, P, d // P, P).transpose(0, 3, 2, 1)
    )


def _pack_acts(x_sign8: np.ndarray) -> np.ndarray:
    """[B, D] fp8 sign matrix -> rhs tiles [P(d_part), D/P, B]."""
    b, d = x_sign8.shape
    return np.ascontiguousarray(x_sign8.T.reshape(d // P, P, b).transpose(1, 0, 2))


def _prep_inputs(x, W1, g1, b1, W2, g2, b2, W3, g3, b3, Wout, b_c, c_pad):
    xs = np.sign(x).astype(FP8)
    a0 = _pack_acts(xs)  # [P, D/P, B]
    ws = [_pack_weight(np.sign(w).astype(FP8)) for w in (W1, W2, W3)]
    c, hdim = Wout.shape
    wo8 = np.zeros((c_pad, hdim), FP8)
    wo8[:c] = np.sign(Wout).astype(FP8)
    wo = _pack_weight(wo8)
    gb = np.stack([g1, b1, g2, b2, g3, b3]).astype(np.float32)  # [6, H]
    hdim2 = gb.shape[1]
    gb = np.ascontiguousarray(gb.reshape(6, hdim2 // P, P).transpose(0, 2, 1))

    n_cores = a0.shape[2] // b_c
    in_maps = [
        {
            "a0": np.ascontiguousarray(a0[:, :, i * b_c : (i + 1) * b_c]),
            "w1": ws[0],
            "w2": ws[1],
            "w3": ws[2],
            "wo": wo,
            "gb": gb,
        }
        for i in range(n_cores)
    ]
    return in_maps


def _assemble_output(results, b_c, c: int) -> np.ndarray:
    """Per-core outT [P, c_pad/P, b_c] -> full [B, C] fp32."""
    blocks = []
    for r in results:
        o = r["outT"]  # [P, fo_n, b_c]
        blocks.append(o.transpose(1, 0, 2).reshape(-1, b_c))  # [c_pad, b_c]
    full = np.concatenate(blocks, axis=1)  # [c_pad, B]
    return np.ascontiguousarray(full[:c].T)


_NC_CACHE: dict = {}


def _get_nc(key, **kw):
    if key not in _NC_CACHE:
        _NC_CACHE[key] = build_kernel(**kw)
    return _NC_CACHE[key]


def kernel(x, W1, g1, b1, W2, g2, b2, W3, g3, b3, Wout):
    from concourse.bass_utils import run_bass_kernel_spmd

    b, d = x.shape
    hdim = W1.shape[0]
    c = Wout.shape[0]
    b_c = b // N_CORES
    c_pad = (c + P - 1) // P * P

    nc = _get_nc(
        (b_c, d, hdim, c_pad), b_c=b_c, d=d, h=hdim, c_pad=c_pad, n_cores=N_CORES
    )
    in_maps = _prep_inputs(
        x, W1, g1, b1, W2, g2, b2, W3, g3, b3, Wout, b_c, c_pad
    )
    res = run_bass_kernel_spmd(nc, in_maps, core_ids=list(range(N_CORES)))
    return _assemble_output(res.results, b_c, c)


def _np_reference(x, W1, g1, b1, W2, g2, b2, W3, g3, b3, Wout):
    """Inline numpy oracle for self-testing (mirrors the jax reference)."""
    h = np.sign(x).astype(np.float64)
    for W, g, bb in ((W1, g1, b1), (W2, g2, b2), (W3, g3, b3)):
        h = h @ np.sign(W).T.astype(np.float64)
        mean = h.mean(axis=0)
        var = h.var(axis=0)
        h = g * (h - mean) / np.sqrt(var + BN_EPS) + bb
        h = np.sign(h)
    return (h @ np.sign(Wout).T.astype(np.float64)).astype(np.float32)


def _selftest():
    rng = np.random.default_rng(1)
    b, d, hdim, c = 4096, 512, 512, 192
    x = rng.standard_normal((b, d)).astype(np.float32)
    W1 = (rng.standard_normal((hdim, d)) * 0.02).astype(np.float32)
    W2 = (rng.standard_normal((hdim, hdim)) * 0.02).astype(np.float32)
    W3 = (rng.standard_normal((hdim, hdim)) * 0.02).astype(np.float32)
    Wout = (rng.standard_normal((c, hdim)) * 0.02).astype(np.float32)
    g = np.ones(hdim, np.float32)
    bb = np.zeros(hdim, np.float32)

    got = kernel(x, W1, g, bb, W2, g, bb, W3, g, bb, Wout)
    want = _np_reference(x, W1, g, bb, W2, g, bb, W3, g, bb, Wout)
    diff = np.abs(got - want)
    denom = max(1e-9, np.abs(want).max())
    print(f"selftest: shape={got.shape} max_abs_err={diff.max()} "
          f"rel={diff.max() / denom:.3e} mismatches={(diff != 0).sum()}")


if __name__ == "__main__":
    _selftest()
